# revision 8
# baseline (speedup 1.0000x reference)
"""Trainium2 Bass kernel for sliding-window causal GQA self-attention.

Problem: B=2, S=2048, E=2048, H=16 q-heads, KV=4 kv-heads, D=128, WIN=1024.
Sharding: 8 cores = (batch b in 2) x (kv-group g in 4). Each core computes
4 q-heads + 1 kv-head for one batch and produces a partial output
[S, E] = attn_out_heads @ Wo_cols.T ; host sums the 4 partials per batch.

On-device layout strategy (per core):
  - x is passed TRANSPOSED (xT[e, s]) so Q/K/V projections contract e on
    partitions with zero on-device transposes.
  - q, k are kept transposed [d, s]; scores are computed transposed
    [k, q] so PV needs attn in [k, q] (native) and v in natural [s, d]
    (made with 16 PE transposes).
  - softmax/RMS reductions over the partition axis use ones-matmuls;
    partition-broadcast of per-column scalars uses rank-1 f32r matmuls.
  - RoPE half-rotation (cross-partition swap) via SBUF->SBUF DMA.
  - sliding-window/causal masks multiply exp(scores) by host-made 0/1
    masks (slices of two [128, 896] ramp masks).
"""

import os
import sys
from contextlib import ExitStack

for _p in ("/opt/trn_rl_repo", "/root/.axon_site/_ro/trn_rl_repo"):
    if os.path.isdir(_p) and _p not in sys.path:
        sys.path.insert(0, _p)

import numpy as np
import ml_dtypes

import concourse.bass as bass
import concourse.mybir as mybir
from concourse import tile
from concourse.bass_utils import run_bass_kernel_spmd

F32 = mybir.dt.float32
BF16 = mybir.dt.bfloat16
F32R = mybir.dt.float32r
AF = mybir.ActivationFunctionType
NP_BF16 = ml_dtypes.bfloat16

H, KV, D, WIN = 16, 4, 128, 1024
B, S, E = 2, 2048, 2048
HPG = H // KV          # q heads per core = 4
DQ = HPG * D           # 512
EPS = float(np.finfo(np.float32).eps)
SCALE = float(D) ** -0.5
NE = E // 128          # 16 e-tiles
NSC = S // 512         # 4 s-chunks
NST = S // 128         # 16 s-subtiles
INV_D = 1.0 / D


def _split_excess_waits(nc, limit=1):
    """walrus rejects >limit sem waits on one instruction (the TileContext
    tail drain gets one wait per outstanding sem). Hoist excess waits onto
    single-wait EventSemaphore instructions inserted just before."""
    cnt = 0
    for bbh in nc.bb_map.values():
        bb = bbh.bb
        new = []
        for inst in bb.instructions:
            si = inst.sync_info
            if si is not None and si.on_wait and len(si.on_wait) > limit:
                waits = list(si.on_wait)
                excess, keep = waits[:-limit], waits[-limit:]
                for i in range(0, len(excess), limit):
                    chunk = excess[i : i + limit]
                    cnt += 1
                    ev = mybir.InstEventSemaphore(
                        name=f"splitwait-{cnt}",
                        engine=inst.engine,
                        ins=[],
                        outs=[],
                        sync_info=mybir.SyncInfo(on_wait=chunk, on_update=[]),
                    )
                    nc.register_instruction(ev, overwrite=True)
                    new.append(ev)
                si.on_wait = keep
            new.append(inst)
        bb.instructions = new
    return cnt


def _k_tiles(qt):
    qs = qt * 512
    return list(range(max(0, qs - WIN), qs + 512, 128))


def _mask_slice(mc, mw, qt, k0):
    """Return the 0/1 [128, 512] mask AP for score tile (qt, k0), or None."""
    off = k0 - qt * 512
    if off >= 0:  # causal edge (tile touches/overlaps the diagonal)
        lo = 384 - off
        return mc[:, lo : lo + 512]
    if off <= -(WIN - 384):  # window edge (off <= -640)
        woff = off + WIN
        lo = 384 - woff
        return mw[:, lo : lo + 512]
    return None


def build_nc():
    nc = bass.Bass("TRN2", target_bir_lowering=False, debug=False)

    xT = nc.dram_tensor("xT", [E, S], BF16, kind="ExternalInput").ap()
    wqT = nc.dram_tensor("wqT", [E, DQ], BF16, kind="ExternalInput").ap()
    wkT = nc.dram_tensor("wkT", [E, D], BF16, kind="ExternalInput").ap()
    wvT = nc.dram_tensor("wvT", [E, D], BF16, kind="ExternalInput").ap()
    woT = nc.dram_tensor("woT", [DQ, E], BF16, kind="ExternalInput").ap()
    cs2 = nc.dram_tensor("cs2", [128, S], F32, kind="ExternalInput").ap()
    ss2 = nc.dram_tensor("ss2", [128, S], F32, kind="ExternalInput").ap()
    maskc = nc.dram_tensor("maskc", [128, 896], BF16, kind="ExternalInput").ap()
    maskw = nc.dram_tensor("maskw", [128, 896], BF16, kind="ExternalInput").ap()
    onesk = nc.dram_tensor("onesk", [128, 1], BF16, kind="ExternalInput").ap()
    onesb = nc.dram_tensor("onesb", [1, 128], F32R, kind="ExternalInput").ap()
    ident = nc.dram_tensor("ident", [128, 128], BF16, kind="ExternalInput").ap()
    out = nc.dram_tensor("out", [S, E], F32, kind="ExternalOutput").ap()

    with tile.TileContext(nc) as tc, ExitStack() as ctx:
        # ---- pools ----
        pin = ctx.enter_context(tc.tile_pool(name="pin", bufs=1))   # persistent inputs
        pq = ctx.enter_context(tc.tile_pool(name="pq", bufs=1))     # qT_n tiles
        pk = ctx.enter_context(tc.tile_pool(name="pk", bufs=1))      # kT_n tiles
        pvt = ctx.enter_context(tc.tile_pool(name="pvt", bufs=1))    # vT tiles
        pv = ctx.enter_context(tc.tile_pool(name="pv", bufs=1))     # v natural tiles
        pat = ctx.enter_context(tc.tile_pool(name="pat", bufs=1))   # attn_outT tiles
        pwork = ctx.enter_context(tc.tile_pool(name="pwork", bufs=2))  # rope Y
        pwork2 = ctx.enter_context(tc.tile_pool(name="pwork2", bufs=2))  # rope Ysw
        pwork3 = ctx.enter_context(tc.tile_pool(name="pwork3", bufs=2))  # rope R
        psq = ctx.enter_context(tc.tile_pool(name="psq", bufs=2))    # squared
        pexp = ctx.enter_context(tc.tile_pool(name="pexp", bufs=3))  # exp tiles
        psm = ctx.enter_context(tc.tile_pool(name="psm", bufs=4))    # small [1,512]
        post = ctx.enter_context(tc.tile_pool(name="post", bufs=3))  # out staging
        # PSUM pools (8 banks total)
        psP = ctx.enter_context(tc.tile_pool(name="psP", bufs=2, space="PSUM"))
        psO = ctx.enter_context(tc.tile_pool(name="psO", bufs=2, space="PSUM"))
        psS = ctx.enter_context(tc.tile_pool(name="psS", bufs=2, space="PSUM"))
        psB = ctx.enter_context(tc.tile_pool(name="psB", bufs=2, space="PSUM"))

        # ---- load persistent inputs ----
        def load(src, shape, dtype, tag):
            t = pin.tile(shape, dtype, tag=tag, name=tag)
            nc.sync.dma_start(t[:], src)
            return t

        xt = [load(xT[i * 128 : (i + 1) * 128, :], [128, S], BF16, f"xt{i}") for i in range(NE)]
        wq = [load(wqT[i * 128 : (i + 1) * 128, :], [128, DQ], BF16, f"wq{i}") for i in range(NE)]
        wk = [load(wkT[i * 128 : (i + 1) * 128, :], [128, D], BF16, f"wk{i}") for i in range(NE)]
        wv = [load(wvT[i * 128 : (i + 1) * 128, :], [128, D], BF16, f"wv{i}") for i in range(NE)]
        wo = [load(woT[h * 128 : (h + 1) * 128, :], [128, E], BF16, f"wo{h}") for h in range(HPG)]
        cst = load(cs2[:, :], [128, S], F32, "cst")
        sst = load(ss2[:, :], [128, S], F32, "sst")
        mc = load(maskc[:, :], [128, 896], BF16, "mc")
        mw = load(maskw[:, :], [128, 896], BF16, "mw")
        onk = load(onesk[:, :], [128, 1], BF16, "onk")
        onb = load(onesb[:, :], [1, 128], F32R, "onb")
        idt = load(ident[:, :], [128, 128], BF16, "idt")

        mc_ap = mc[:]
        mw_ap = mw[:]

        epsb = pin.tile([1, 1], F32, tag="epsb", name="epsb")
        nc.vector.memset(epsb[:], EPS)

        # ---- rope + rms-norm: psum P [128(d), 512(s)] -> dst bf16 slice ----
        def rope_rms(P, sc, dst_ap):
            cols = slice(sc * 512, (sc + 1) * 512)
            Y = pwork.tile([128, 512], F32, tag="ropey")
            nc.vector.tensor_copy(Y[:], P[:])
            Ysw = pwork2.tile([128, 512], F32, tag="ropesw")
            nc.sync.dma_start(Ysw[0:64, :], Y[64:128, :])
            nc.sync.dma_start(Ysw[64:128, :], Y[0:64, :])
            R = pwork3.tile([128, 512], F32, tag="roper")
            nc.vector.tensor_mul(R[:], Y[:], cst[:, cols])
            nc.vector.tensor_mul(Y[:], Ysw[:], sst[:, cols])
            nc.vector.tensor_add(R[:], R[:], Y[:])
            # rms: rs = 1/sqrt(mean(R^2) + eps) per column, bcast over partitions
            SQ = psq.tile([128, 512], BF16, tag="sq")
            nc.scalar.square(SQ[:], R[:])
            sums = psS.tile([1, 512], F32, tag="sums")
            nc.tensor.matmul(sums[:], onk[:], SQ[:], start=True, stop=True)
            sqs = psm.tile([1, 512], F32, tag="sqs")
            nc.scalar.activation(sqs[:], sums[:], AF.Sqrt, bias=epsb[:], scale=INV_D)
            rs = psm.tile([1, 512], F32R, tag="rs", name="rs")
            with nc.allow_low_precision(reason="f32r broadcast scale"):
                nc.vector.reciprocal(rs[:], sqs[:])
            BC = psB.tile([128, 512], F32, tag="bc")
            nc.tensor.matmul(BC[:], onb[:], rs[:], start=True, stop=True)
            nc.vector.tensor_mul(dst_ap, R[:], BC[:])

        # ---- K projection + rope + rms ----
        kt_n = [pk.tile([128, 512], BF16, tag=f"ktn{sc}", name=f"ktn{sc}") for sc in range(NSC)]
        for sc in range(NSC):
            P = psP.tile([128, 512], F32, tag="acc")
            for et in range(NE):
                nc.tensor.matmul(
                    P[:], wk[et][:], xt[et][:, sc * 512 : (sc + 1) * 512],
                    start=(et == 0), stop=(et == NE - 1),
                )
            rope_rms(P, sc, kt_n[sc][:])

        # ---- V projection (transposed), then PE-transpose to natural ----
        vt_sb = [pvt.tile([128, 512], BF16, tag=f"vt{sc}", name=f"vt{sc}") for sc in range(NSC)]
        for sc in range(NSC):
            P = psP.tile([128, 512], F32, tag="acc")
            for et in range(NE):
                nc.tensor.matmul(
                    P[:], wv[et][:], xt[et][:, sc * 512 : (sc + 1) * 512],
                    start=(et == 0), stop=(et == NE - 1),
                )
            nc.vector.tensor_copy(vt_sb[sc][:], P[:])
        v_nat = [pv.tile([128, 128], BF16, tag=f"vn{st}", name=f"vn{st}") for st in range(NST)]
        for st in range(NST):
            sc, j = st // 4, st % 4
            TP = psB.tile([128, 128], BF16, tag="bc")
            nc.tensor.transpose(TP[:], vt_sb[sc][:, j * 128 : (j + 1) * 128], idt[:])
            nc.vector.tensor_copy(v_nat[st][:], TP[:])

        # ---- Q projection + rope + rms ----
        qt_n = {}
        for h in range(HPG):
            for sc in range(NSC):
                qt_n[(h, sc)] = pq.tile([128, 512], BF16, tag=f"qtn{h}_{sc}", name=f"qtn{h}_{sc}")
        for h in range(HPG):
            for sc in range(NSC):
                P = psP.tile([128, 512], F32, tag="acc")
                for et in range(NE):
                    nc.tensor.matmul(
                        P[:], wq[et][:, h * 128 : (h + 1) * 128],
                        xt[et][:, sc * 512 : (sc + 1) * 512],
                        start=(et == 0), stop=(et == NE - 1),
                    )
                rope_rms(P, sc, qt_n[(h, sc)][:])

        # ---- attention ----
        at_n = {}
        for h in range(HPG):
            for qt in range(NSC):
                at_n[(h, qt)] = pat.tile([128, 512], BF16, tag=f"at{h}_{qt}", name=f"at{h}_{qt}")
        for qt in range(NSC):
            kts = _k_tiles(qt)
            last = len(kts) - 1
            for h in range(HPG):
                O = psO.tile([128, 512], F32, tag="pv")
                sums = psS.tile([1, 512], F32, tag="sums")
                for i, k0 in enumerate(kts):
                    sc_ps = psP.tile([128, 512], F32, tag="acc")
                    ksc, kj = k0 // 512, k0 % 512
                    nc.tensor.matmul(
                        sc_ps[:], kt_n[ksc][:, kj : kj + 128], qt_n[(h, qt)][:],
                        start=True, stop=True,
                    )
                    ex = pexp.tile([128, 512], BF16, tag="exp")
                    nc.scalar.activation(ex[:], sc_ps[:], AF.Exp, scale=SCALE)
                    msk = _mask_slice(mc_ap, mw_ap, qt, k0)
                    if msk is not None:
                        nc.vector.tensor_mul(ex[:], ex[:], msk)
                    nc.tensor.matmul(
                        O[:], v_nat[k0 // 128][:], ex[:],
                        start=(i == 0), stop=(i == last),
                    )
                    nc.tensor.matmul(
                        sums[:], onk[:], ex[:], start=(i == 0), stop=(i == last)
                    )
                rs = psm.tile([1, 512], F32R, tag="rs", name="rs")
                with nc.allow_low_precision(reason="f32r broadcast scale"):
                    nc.vector.reciprocal(rs[:], sums[:])
                BC = psB.tile([128, 512], F32, tag="bc")
                nc.tensor.matmul(BC[:], onb[:], rs[:], start=True, stop=True)
                BCs = pwork.tile([128, 512], F32, tag="ropey", name="bcs")
                nc.scalar.activation(BCs[:], BC[:], AF.Copy)
                nc.vector.tensor_mul(at_n[(h, qt)][:], O[:], BCs[:])

        # ---- final projection: out[s, e] = sum_h attn_outT_h.T @ woT_h ----
        for st in range(NST):
            qt, jj = st // 4, st % 4
            for ec in range(NSC):
                o_ps = psP.tile([128, 512], F32, tag="acc")
                for h in range(HPG):
                    nc.tensor.matmul(
                        o_ps[:],
                        at_n[(h, qt)][:, jj * 128 : (jj + 1) * 128],
                        wo[h][:, ec * 512 : (ec + 1) * 512],
                        start=(h == 0), stop=(h == HPG - 1),
                    )
                OS = post.tile([128, 512], F32, tag="ostage")
                nc.vector.tensor_copy(OS[:], o_ps[:])
                nc.sync.dma_start(
                    out[st * 128 : (st + 1) * 128, ec * 512 : (ec + 1) * 512], OS[:]
                )

    _split_excess_waits(nc)
    return nc


_NC_CACHE = None


def _get_nc():
    global _NC_CACHE
    if _NC_CACHE is None:
        _NC_CACHE = build_nc()
    return _NC_CACHE


def _prep_in_maps(x, cos, sin, Wq, Wk, Wv, Wo):
    cosT = np.ascontiguousarray(cos[0, 0].T).astype(np.float32)  # [64, S]
    sinT = np.ascontiguousarray(sin[0, 0].T).astype(np.float32)
    cs2 = np.concatenate([cosT, cosT], axis=0)                   # [128, S]
    ss2 = np.concatenate([sinT, -sinT], axis=0)
    r = np.arange(128, dtype=np.int64)[:, None]
    c = np.arange(896, dtype=np.int64)[None, :]
    maskc = (c >= r + 384).astype(NP_BF16)
    maskw = (c < r + 384).astype(NP_BF16)
    onesk = np.ones((128, 1), dtype=NP_BF16)
    onesb = np.ones((1, 128), dtype=np.float32)
    ident = np.eye(128, dtype=NP_BF16)

    in_maps = []
    for core in range(8):
        b, g = core // 4, core % 4
        xTb = np.ascontiguousarray(x[b].T).astype(NP_BF16)
        wqT = np.ascontiguousarray(Wq[g * DQ : (g + 1) * DQ].T).astype(NP_BF16)
        wkT = np.ascontiguousarray(Wk[g * D : (g + 1) * D].T).astype(NP_BF16)
        wvT = np.ascontiguousarray(Wv[g * D : (g + 1) * D].T).astype(NP_BF16)
        woT = np.ascontiguousarray(Wo[:, g * DQ : (g + 1) * DQ].T).astype(NP_BF16)
        in_maps.append(
            dict(
                xT=xTb, wqT=wqT, wkT=wkT, wvT=wvT, woT=woT,
                cs2=cs2, ss2=ss2, maskc=maskc, maskw=maskw,
                onesk=onesk, onesb=onesb, ident=ident,
            )
        )
    return in_maps


def run(x, cos, sin, Wq, Wk, Wv, Wo, trace=False):
    nc = _get_nc()
    in_maps = _prep_in_maps(
        np.asarray(x), np.asarray(cos), np.asarray(sin),
        np.asarray(Wq), np.asarray(Wk), np.asarray(Wv), np.asarray(Wo),
    )
    kw = {}
    if trace:
        _install_ntff_hook()
        kw["trace"] = True
    res = run_bass_kernel_spmd(nc, in_maps, list(range(8)), **kw)
    out = np.zeros((B, S, E), dtype=np.float32)
    for core in range(8):
        out[core // 4] += res.results[core]["out"].astype(np.float32)
    return out, res


def kernel(x, cos, sin, Wq, Wk, Wv, Wo):
    out, _ = run(x, cos, sin, Wq, Wk, Wv, Wo, trace=False)
    return out


def _install_ntff_hook():
    """Register the NTFF profile hook missing from this image's antenv."""
    import types
    import ctypes
    import contextlib

    if "antenv.axon_hooks" in sys.modules:
        return
    so_path = "/opt/axon/libaxon_pjrt.so"

    lib = ctypes.CDLL(so_path)
    if not hasattr(lib, "axon_start_nrt_profile"):
        return
    lib.axon_start_nrt_profile.argtypes = [
        ctypes.POINTER(ctypes.c_int64),
        ctypes.c_size_t,
    ]
    lib.axon_start_nrt_profile.restype = ctypes.c_int64
    lib.axon_stop_nrt_profile.argtypes = [ctypes.c_char_p]
    lib.axon_stop_nrt_profile.restype = ctypes.c_int64

    @contextlib.contextmanager
    def _hook(output_dir, device_ids):
        import jax

        jax.devices()
        if device_ids:
            ids = (ctypes.c_int64 * len(device_ids))(*device_ids)
            rc = lib.axon_start_nrt_profile(ids, len(device_ids))
        else:
            rc = lib.axon_start_nrt_profile(None, 0)
        if rc != 0:
            raise RuntimeError(f"axon_start_nrt_profile rc={rc}")
        try:
            yield
        finally:
            n = lib.axon_stop_nrt_profile(str(output_dir).encode())
            print(f"ntff profile: {n} file(s) -> {output_dir}", file=sys.stderr)

    mod = types.ModuleType("antenv.axon_hooks")
    mod.get_axon_ntff_profile_hook = lambda: _hook
    mod.set_axon_ntff_profile_hook = lambda h: None
    import antenv

    sys.modules["antenv.axon_hooks"] = mod
    antenv.axon_hooks = mod


# revision 9
# speedup vs baseline: 1.0024x; 1.0024x over previous
"""Trainium2 Bass kernel for sliding-window causal GQA self-attention.

Problem: B=2, S=2048, E=2048, H=16 q-heads, KV=4 kv-heads, D=128, WIN=1024.
Sharding: 8 cores = (batch b in 2) x (kv-group g in 4). Each core computes
4 q-heads + 1 kv-head for one batch and produces a partial output
[S, E] = attn_out_heads @ Wo_cols.T ; host sums the 4 partials per batch.

On-device layout strategy (per core):
  - x is passed TRANSPOSED (xT[e, s]) so Q/K/V projections contract e on
    partitions with zero on-device transposes.
  - q, k are kept transposed [d, s]; scores are computed transposed
    [k, q] so PV needs attn in [k, q] (native) and v in natural [s, d]
    (made with 16 PE transposes).
  - softmax/RMS reductions over the partition axis use ones-matmuls;
    partition-broadcast of per-column scalars uses rank-1 f32r matmuls.
  - RoPE half-rotation (cross-partition swap) via SBUF->SBUF DMA.
  - sliding-window/causal masks multiply exp(scores) by host-made 0/1
    masks (slices of two [128, 896] ramp masks).
"""

import os
import sys
from contextlib import ExitStack

for _p in ("/opt/trn_rl_repo", "/root/.axon_site/_ro/trn_rl_repo"):
    if os.path.isdir(_p) and _p not in sys.path:
        sys.path.insert(0, _p)

import numpy as np
import ml_dtypes

import concourse.bass as bass
import concourse.mybir as mybir
from concourse import tile
from concourse.bass_utils import run_bass_kernel_spmd

F32 = mybir.dt.float32
BF16 = mybir.dt.bfloat16
F32R = mybir.dt.float32r
AF = mybir.ActivationFunctionType
NP_BF16 = ml_dtypes.bfloat16

H, KV, D, WIN = 16, 4, 128, 1024
B, S, E = 2, 2048, 2048
HPG = H // KV          # q heads per core = 4
DQ = HPG * D           # 512
EPS = float(np.finfo(np.float32).eps)
SCALE = float(D) ** -0.5
NE = E // 128          # 16 e-tiles
NSC = S // 512         # 4 s-chunks
NST = S // 128         # 16 s-subtiles
INV_D = 1.0 / D


def _split_excess_waits(nc, limit=1):
    """walrus rejects >limit sem waits on one instruction (the TileContext
    tail drain gets one wait per outstanding sem). Hoist excess waits onto
    single-wait EventSemaphore instructions inserted just before."""
    cnt = 0
    for bbh in nc.bb_map.values():
        bb = bbh.bb
        new = []
        for inst in bb.instructions:
            si = inst.sync_info
            if si is not None and si.on_wait and len(si.on_wait) > limit:
                waits = list(si.on_wait)
                excess, keep = waits[:-limit], waits[-limit:]
                for i in range(0, len(excess), limit):
                    chunk = excess[i : i + limit]
                    cnt += 1
                    ev = mybir.InstEventSemaphore(
                        name=f"splitwait-{cnt}",
                        engine=inst.engine,
                        ins=[],
                        outs=[],
                        sync_info=mybir.SyncInfo(on_wait=chunk, on_update=[]),
                    )
                    nc.register_instruction(ev, overwrite=True)
                    new.append(ev)
                si.on_wait = keep
            new.append(inst)
        bb.instructions = new
    return cnt


def _k_tiles(qt):
    qs = qt * 512
    return list(range(max(0, qs - WIN), qs + 512, 128))


def _mask_slice(mc, mw, qt, k0):
    """Return the 0/1 [128, 512] mask AP for score tile (qt, k0), or None."""
    off = k0 - qt * 512
    if off >= 0:  # causal edge (tile touches/overlaps the diagonal)
        lo = 384 - off
        return mc[:, lo : lo + 512]
    if off <= -(WIN - 384):  # window edge (off <= -640)
        woff = off + WIN
        lo = 384 - woff
        return mw[:, lo : lo + 512]
    return None


def build_nc():
    nc = bass.Bass("TRN2", target_bir_lowering=False, debug=False)

    xT = nc.dram_tensor("xT", [E, S], BF16, kind="ExternalInput").ap()
    wqT = nc.dram_tensor("wqT", [E, DQ], BF16, kind="ExternalInput").ap()
    wkT = nc.dram_tensor("wkT", [E, D], BF16, kind="ExternalInput").ap()
    wvT = nc.dram_tensor("wvT", [E, D], BF16, kind="ExternalInput").ap()
    woT = nc.dram_tensor("woT", [DQ, E], BF16, kind="ExternalInput").ap()
    cs2 = nc.dram_tensor("cs2", [128, S], F32, kind="ExternalInput").ap()
    ss2 = nc.dram_tensor("ss2", [128, S], F32, kind="ExternalInput").ap()
    maskc = nc.dram_tensor("maskc", [128, 896], BF16, kind="ExternalInput").ap()
    maskw = nc.dram_tensor("maskw", [128, 896], BF16, kind="ExternalInput").ap()
    onesk = nc.dram_tensor("onesk", [128, 1], BF16, kind="ExternalInput").ap()
    onesb = nc.dram_tensor("onesb", [1, 128], F32R, kind="ExternalInput").ap()
    ident = nc.dram_tensor("ident", [128, 128], BF16, kind="ExternalInput").ap()
    out = nc.dram_tensor("out", [S, E], F32, kind="ExternalOutput").ap()

    with tile.TileContext(nc) as tc, ExitStack() as ctx:
        # ---- pools ----
        pin = ctx.enter_context(tc.tile_pool(name="pin", bufs=1))   # persistent inputs
        pq = ctx.enter_context(tc.tile_pool(name="pq", bufs=1))     # qT_n tiles
        pk = ctx.enter_context(tc.tile_pool(name="pk", bufs=1))      # kT_n tiles
        pvt = ctx.enter_context(tc.tile_pool(name="pvt", bufs=1))    # vT tiles
        pv = ctx.enter_context(tc.tile_pool(name="pv", bufs=1))     # v natural tiles
        pat = ctx.enter_context(tc.tile_pool(name="pat", bufs=1))   # attn_outT tiles
        pwork = ctx.enter_context(tc.tile_pool(name="pwork", bufs=2))  # rope Y
        pwork2 = ctx.enter_context(tc.tile_pool(name="pwork2", bufs=2))  # rope Ysw
        pwork3 = ctx.enter_context(tc.tile_pool(name="pwork3", bufs=2))  # rope R
        psq = ctx.enter_context(tc.tile_pool(name="psq", bufs=2))    # squared
        pexp = ctx.enter_context(tc.tile_pool(name="pexp", bufs=3))  # exp tiles
        psm = ctx.enter_context(tc.tile_pool(name="psm", bufs=4))    # small [1,512]
        post = ctx.enter_context(tc.tile_pool(name="post", bufs=3))  # out staging
        # PSUM pools (8 banks total)
        psP = ctx.enter_context(tc.tile_pool(name="psP", bufs=2, space="PSUM"))
        psO = ctx.enter_context(tc.tile_pool(name="psO", bufs=2, space="PSUM"))
        psS = ctx.enter_context(tc.tile_pool(name="psS", bufs=2, space="PSUM"))
        psB = ctx.enter_context(tc.tile_pool(name="psB", bufs=2, space="PSUM"))

        # ---- load persistent inputs ----
        def load(src, shape, dtype, tag):
            t = pin.tile(shape, dtype, tag=tag, name=tag)
            nc.sync.dma_start(t[:], src)
            return t

        xt = [load(xT[i * 128 : (i + 1) * 128, :], [128, S], BF16, f"xt{i}") for i in range(NE)]
        wq = [load(wqT[i * 128 : (i + 1) * 128, :], [128, DQ], BF16, f"wq{i}") for i in range(NE)]
        wk = [load(wkT[i * 128 : (i + 1) * 128, :], [128, D], BF16, f"wk{i}") for i in range(NE)]
        wv = [load(wvT[i * 128 : (i + 1) * 128, :], [128, D], BF16, f"wv{i}") for i in range(NE)]
        wo = [load(woT[h * 128 : (h + 1) * 128, :], [128, E], BF16, f"wo{h}") for h in range(HPG)]
        cst = load(cs2[:, :], [128, S], F32, "cst")
        sst = load(ss2[:, :], [128, S], F32, "sst")
        mc = load(maskc[:, :], [128, 896], BF16, "mc")
        mw = load(maskw[:, :], [128, 896], BF16, "mw")
        onk = load(onesk[:, :], [128, 1], BF16, "onk")
        onb = load(onesb[:, :], [1, 128], F32R, "onb")
        idt = load(ident[:, :], [128, 128], BF16, "idt")

        mc_ap = mc[:]
        mw_ap = mw[:]

        epsb = pin.tile([1, 1], F32, tag="epsb", name="epsb")
        nc.vector.memset(epsb[:], EPS)

        # ---- rope + rms-norm: psum P [128(d), 512(s)] -> dst bf16 slice ----
        def rope_rms(P, sc, dst_ap):
            cols = slice(sc * 512, (sc + 1) * 512)
            Y = pwork.tile([128, 512], F32, tag="ropey")
            nc.vector.tensor_copy(Y[:], P[:])
            Ysw = pwork2.tile([128, 512], F32, tag="ropesw")
            nc.sync.dma_start(Ysw[0:64, :], Y[64:128, :])
            nc.sync.dma_start(Ysw[64:128, :], Y[0:64, :])
            R = pwork3.tile([128, 512], F32, tag="roper")
            nc.vector.tensor_mul(R[:], Y[:], cst[:, cols])
            nc.vector.tensor_mul(Y[:], Ysw[:], sst[:, cols])
            nc.vector.tensor_add(R[:], R[:], Y[:])
            # rms: rs = 1/sqrt(mean(R^2) + eps) per column, bcast over partitions
            SQ = psq.tile([128, 512], BF16, tag="sq")
            nc.scalar.square(SQ[:], R[:])
            sums = psS.tile([1, 512], F32, tag="sums")
            nc.tensor.matmul(sums[:], onk[:], SQ[:], start=True, stop=True)
            sqs = psm.tile([1, 512], F32, tag="sqs")
            nc.scalar.activation(sqs[:], sums[:], AF.Sqrt, bias=epsb[:], scale=INV_D)
            rs = psm.tile([1, 512], F32R, tag="rs", name="rs")
            with nc.allow_low_precision(reason="f32r broadcast scale"):
                nc.vector.reciprocal(rs[:], sqs[:])
            BC = psB.tile([128, 512], F32, tag="bc")
            nc.tensor.matmul(BC[:], onb[:], rs[:], start=True, stop=True)
            nc.vector.tensor_mul(dst_ap, R[:], BC[:])

        # ---- K projection + rope + rms ----
        kt_n = [pk.tile([128, 512], BF16, tag=f"ktn{sc}", name=f"ktn{sc}") for sc in range(NSC)]
        for sc in range(NSC):
            P = psP.tile([128, 512], F32, tag="acc")
            for et in range(NE):
                nc.tensor.matmul(
                    P[:], wk[et][:], xt[et][:, sc * 512 : (sc + 1) * 512],
                    start=(et == 0), stop=(et == NE - 1),
                )
            rope_rms(P, sc, kt_n[sc][:])

        # ---- V projection (transposed), then PE-transpose to natural ----
        vt_sb = [pvt.tile([128, 512], BF16, tag=f"vt{sc}", name=f"vt{sc}") for sc in range(NSC)]
        for sc in range(NSC):
            P = psP.tile([128, 512], F32, tag="acc")
            for et in range(NE):
                nc.tensor.matmul(
                    P[:], wv[et][:], xt[et][:, sc * 512 : (sc + 1) * 512],
                    start=(et == 0), stop=(et == NE - 1),
                )
            nc.vector.tensor_copy(vt_sb[sc][:], P[:])
        v_nat = [pv.tile([128, 128], BF16, tag=f"vn{st}", name=f"vn{st}") for st in range(NST)]
        for st in range(NST):
            sc, j = st // 4, st % 4
            TP = psB.tile([128, 128], BF16, tag="bc")
            nc.tensor.transpose(TP[:], vt_sb[sc][:, j * 128 : (j + 1) * 128], idt[:])
            nc.vector.tensor_copy(v_nat[st][:], TP[:])

        # ---- Q projection + rope + rms ----
        qt_n = {}
        for h in range(HPG):
            for sc in range(NSC):
                qt_n[(h, sc)] = pq.tile([128, 512], BF16, tag=f"qtn{h}_{sc}", name=f"qtn{h}_{sc}")
        for h in range(HPG):
            for sc in range(NSC):
                P = psP.tile([128, 512], F32, tag="acc")
                for et in range(NE):
                    nc.tensor.matmul(
                        P[:], wq[et][:, h * 128 : (h + 1) * 128],
                        xt[et][:, sc * 512 : (sc + 1) * 512],
                        start=(et == 0), stop=(et == NE - 1),
                    )
                rope_rms(P, sc, qt_n[(h, sc)][:])

        # ---- attention ----
        at_n = {}
        for h in range(HPG):
            for qt in range(NSC):
                at_n[(h, qt)] = pat.tile([128, 512], BF16, tag=f"at{h}_{qt}", name=f"at{h}_{qt}")
        for qt in range(NSC):
            kts = _k_tiles(qt)
            last = len(kts) - 1
            for h in range(HPG):
                O = psO.tile([128, 512], F32, tag="pv")
                sums = psS.tile([1, 512], F32, tag="sums")
                for i, k0 in enumerate(kts):
                    sc_ps = psP.tile([128, 512], F32, tag="acc")
                    ksc, kj = k0 // 512, k0 % 512
                    nc.tensor.matmul(
                        sc_ps[:], kt_n[ksc][:, kj : kj + 128], qt_n[(h, qt)][:],
                        start=True, stop=True,
                    )
                    ex = pexp.tile([128, 512], BF16, tag="exp")
                    nc.scalar.activation(ex[:], sc_ps[:], AF.Exp, scale=SCALE)
                    msk = _mask_slice(mc_ap, mw_ap, qt, k0)
                    if msk is not None:
                        nc.vector.tensor_mul(ex[:], ex[:], msk)
                    nc.tensor.matmul(
                        O[:], v_nat[k0 // 128][:], ex[:],
                        start=(i == 0), stop=(i == last),
                    )
                    nc.tensor.matmul(
                        sums[:], onk[:], ex[:], start=(i == 0), stop=(i == last)
                    )
                rs = psm.tile([1, 512], F32R, tag="rs", name="rs")
                with nc.allow_low_precision(reason="f32r broadcast scale"):
                    nc.vector.reciprocal(rs[:], sums[:])
                BC = psB.tile([128, 512], F32, tag="bc")
                nc.tensor.matmul(BC[:], onb[:], rs[:], start=True, stop=True)
                BCs = pwork.tile([128, 512], F32, tag="ropey", name="bcs")
                nc.scalar.activation(BCs[:], BC[:], AF.Copy)
                nc.vector.tensor_mul(at_n[(h, qt)][:], O[:], BCs[:])

        # ---- final projection: out[s, e] = sum_h attn_outT_h.T @ woT_h ----
        for st in range(NST):
            qt, jj = st // 4, st % 4
            for ec in range(NSC):
                o_ps = psP.tile([128, 512], F32, tag="acc")
                for h in range(HPG):
                    nc.tensor.matmul(
                        o_ps[:],
                        at_n[(h, qt)][:, jj * 128 : (jj + 1) * 128],
                        wo[h][:, ec * 512 : (ec + 1) * 512],
                        start=(h == 0), stop=(h == HPG - 1),
                    )
                OS = post.tile([128, 512], F32, tag="ostage")
                nc.vector.tensor_copy(OS[:], o_ps[:])
                nc.sync.dma_start(
                    out[st * 128 : (st + 1) * 128, ec * 512 : (ec + 1) * 512], OS[:]
                )

    _split_excess_waits(nc)
    return nc


_NC_CACHE = None


def _get_nc():
    global _NC_CACHE
    if _NC_CACHE is None:
        _NC_CACHE = build_nc()
    return _NC_CACHE


def _prep_in_maps(x, cos, sin, Wq, Wk, Wv, Wo):
    cosT = np.ascontiguousarray(cos[0, 0].T).astype(np.float32)  # [64, S]
    sinT = np.ascontiguousarray(sin[0, 0].T).astype(np.float32)
    cs2 = np.concatenate([cosT, cosT], axis=0)                   # [128, S]
    ss2 = np.concatenate([sinT, -sinT], axis=0)
    r = np.arange(128, dtype=np.int64)[:, None]
    c = np.arange(896, dtype=np.int64)[None, :]
    maskc = (c >= r + 384).astype(NP_BF16)
    maskw = (c < r + 384).astype(NP_BF16)
    onesk = np.ones((128, 1), dtype=NP_BF16)
    onesb = np.ones((1, 128), dtype=np.float32)
    ident = np.eye(128, dtype=NP_BF16)

    in_maps = []
    for core in range(8):
        b, g = core // 4, core % 4
        xTb = np.ascontiguousarray(x[b].T).astype(NP_BF16)
        wqT = np.ascontiguousarray(Wq[g * DQ : (g + 1) * DQ].T).astype(NP_BF16)
        wkT = np.ascontiguousarray(Wk[g * D : (g + 1) * D].T).astype(NP_BF16)
        wvT = np.ascontiguousarray(Wv[g * D : (g + 1) * D].T).astype(NP_BF16)
        woT = np.ascontiguousarray(Wo[:, g * DQ : (g + 1) * DQ].T).astype(NP_BF16)
        in_maps.append(
            dict(
                xT=xTb, wqT=wqT, wkT=wkT, wvT=wvT, woT=woT,
                cs2=cs2, ss2=ss2, maskc=maskc, maskw=maskw,
                onesk=onesk, onesb=onesb, ident=ident,
            )
        )
    return in_maps


def run(x, cos, sin, Wq, Wk, Wv, Wo, trace=False):
    nc = _get_nc()
    in_maps = _prep_in_maps(
        np.asarray(x), np.asarray(cos), np.asarray(sin),
        np.asarray(Wq), np.asarray(Wk), np.asarray(Wv), np.asarray(Wo),
    )
    kw = {}
    if trace:
        _install_ntff_hook()
        kw["trace"] = True
        kw["tmpdir"] = "/root/problem/prof"
        import shutil
        shutil.rmtree("/root/problem/prof", ignore_errors=True)
        os.makedirs("/root/problem/prof", exist_ok=True)
    res = run_bass_kernel_spmd(nc, in_maps, list(range(8)), **kw)
    out = np.zeros((B, S, E), dtype=np.float32)
    for core in range(8):
        out[core // 4] += res.results[core]["out"].astype(np.float32)
    return out, res


def kernel(x, cos, sin, Wq, Wk, Wv, Wo):
    out, _ = run(x, cos, sin, Wq, Wk, Wv, Wo, trace=False)
    return out


def _install_ntff_hook():
    """Register the NTFF profile hook missing from this image's antenv."""
    import types
    import ctypes
    import contextlib

    if "antenv.axon_hooks" in sys.modules:
        return
    so_path = "/opt/axon/libaxon_pjrt.so"

    lib = ctypes.CDLL(so_path)
    if not hasattr(lib, "axon_start_nrt_profile"):
        return
    lib.axon_start_nrt_profile.argtypes = [
        ctypes.POINTER(ctypes.c_int64),
        ctypes.c_size_t,
    ]
    lib.axon_start_nrt_profile.restype = ctypes.c_int64
    lib.axon_stop_nrt_profile.argtypes = [ctypes.c_char_p]
    lib.axon_stop_nrt_profile.restype = ctypes.c_int64

    @contextlib.contextmanager
    def _hook(output_dir, device_ids):
        import jax

        jax.devices()
        if device_ids:
            ids = (ctypes.c_int64 * len(device_ids))(*device_ids)
            rc = lib.axon_start_nrt_profile(ids, len(device_ids))
        else:
            rc = lib.axon_start_nrt_profile(None, 0)
        if rc != 0:
            raise RuntimeError(f"axon_start_nrt_profile rc={rc}")
        try:
            yield
        finally:
            n = lib.axon_stop_nrt_profile(str(output_dir).encode())
            print(f"ntff profile: {n} file(s) -> {output_dir}", file=sys.stderr)

    mod = types.ModuleType("antenv.axon_hooks")
    mod.get_axon_ntff_profile_hook = lambda: _hook
    mod.set_axon_ntff_profile_hook = lambda h: None
    import antenv

    sys.modules["antenv.axon_hooks"] = mod
    antenv.axon_hooks = mod
